# revision 4
# baseline (speedup 1.0000x reference)
"""Multi-head self-attention TRN2 kernel (B=2, S=2048, D=1024, H=16, DH=64).

Sharding: 2 heads per core across 8 cores (tensor parallel). Each core
computes q/k/v projections for its heads, attention, and a partial output
projection y_c = o_c @ Wo[c*128:(c+1)*128]; the host sums the 8 partials
and adds bo.

Layout strategy (per core):
  - x^T tiles produced on-chip via PE transposes; QKV computed
    feature-major: ftq/ftk = [128 (2 heads x 64), 4096 tokens].
  - scores computed transposed: scoresT[s_j, s_i] = kT.T @ qT with the two
    heads row-packed in the PE array (K=64 each, concurrent).
  - softmax without max-subtraction (scores are bounded ~|3.4|); exp on ACT
    reading PSUM [128, 1024] (both heads), writing fp32r PT tiles.
  - PV uses an augmented v tile [s_j, 65] whose last column is ones, so the
    softmax denominator accumulates in PSUM row 64 for free.
  - 1/den via ACT Ln + Exp(-x); broadcast along partitions via a K=1
    ones-matmul; normalization on DVE.
  - all matmuls in float32r (full PE rate at N=512).
"""

import time
import numpy as np

import concourse.bass as bass
import concourse.tile as tile
from concourse import mybir
from concourse.bass_utils import run_bass_kernel_spmd
from concourse.masks import make_identity

F32 = mybir.dt.float32
F32R = mybir.dt.float32r

B, S, D, H, DH = 2, 2048, 1024, 16, 64
TOK = B * S          # 4096
NCORES = 8
HPC = H // NCORES    # heads per core = 2
FDIM = 3 * HPC * DH  # 384 packed qkv features per core
NCH = TOK // 512     # token chunks for projection
NDT = D // 128       # contraction d-tiles
NSC = S // 512       # si chunks per batch
NJT = S // 128       # sj tiles per batch


def _split_multi_waits(nc):
    """This toolchain's walrus allows only ONE sync-wait per instruction;
    hoist excess waits onto same-engine NOPs placed immediately before."""
    made = 0
    for bb in nc.main_func.blocks:
        insts = list(bb.instructions)
        out = []
        for ins in insts:
            si = ins.sync_info
            waits = list(si.on_wait) if (si is not None and si.on_wait) else []
            if len(waits) > 1:
                for w in waits[:-1]:
                    bi = nc.engines[ins.engine].nop()
                    nop_inst = bi.ins
                    cur = nc.cur_bb.bb
                    lst = list(cur.instructions)
                    assert lst[-1].name == nop_inst.name
                    lst.pop()
                    cur.instructions = lst
                    nop_inst.sync_info = mybir.SyncInfo(on_wait=[w], on_update=[])
                    out.append(nop_inst)
                    made += 1
                si.on_wait = waits[-1:]
            out.append(ins)
        bb.instructions = out
    return made


def build_bass():
    nc = bass.Bass("TRN2", target_bir_lowering=False, debug=False,
                   num_devices=NCORES)
    x_in = nc.declare_dram_parameter("x", [TOK, D], F32, isOutput=False)
    w_in = nc.declare_dram_parameter("w", [D, FDIM], F32, isOutput=False)
    b_in = nc.declare_dram_parameter("b", [FDIM, 1], F32, isOutput=False)
    wo_in = nc.declare_dram_parameter("wo", [128, D], F32, isOutput=False)
    y_out = nc.declare_dram_parameter("y", [TOK, D], F32, isOutput=True)

    with tile.TileContext(nc) as tc:
        ACT = mybir.ActivationFunctionType
        with tc.tile_pool(name="consts", bufs=1) as consts, \
             tc.tile_pool(name="fts", bufs=1) as fts:
            ident = consts.tile([128, 128], F32)
            make_identity(nc, ident)
            onescol_f = consts.tile([128, 1], F32)
            nc.vector.memset(onescol_f, 1.0)
            ones1 = consts.tile([1, 64], F32R)
            nc.vector.tensor_copy(ones1, onescol_f[0:1, :].broadcast_to([1, 64]))
            bias_sb = consts.tile([128, 3], F32)
            for f in range(3):
                nc.sync.dma_start(out=bias_sb[:, f:f + 1],
                                  in_=b_in[f * 128:(f + 1) * 128, :])
            w_r = consts.tile([128, NDT * FDIM], F32R)  # d-tile-major packed w
            woA_r = consts.tile([64, D], F32R)
            woB_r = consts.tile([64, D], F32R)

            # feature-major activations: [feat, token]
            ftq = fts.tile([128, TOK], F32R)
            ftk = fts.tile([128, TOK], F32R)
            ftv = fts.tile([128, TOK], F32)
            oT_A = fts.tile([64, TOK], F32R)   # normalized per-head outputs
            oT_B = fts.tile([64, TOK], F32R)
            # augmented token-major v tiles: per (b, head): [128, 16*65]
            vall = [[fts.tile([128, NJT * 65], F32R,
                              name=f"vall_{_b}_{_h}", tag=f"vall_{_b}_{_h}")
                     for _h in range(2)]
                    for _b in range(B)]

            # ---- load + round weights ----
            with tc.tile_pool(name="wload", bufs=1) as wload:
                w_f = wload.tile([128, NDT * FDIM], F32)
                for dt in range(NDT):
                    nc.sync.dma_start(
                        out=w_f[:, dt * FDIM:(dt + 1) * FDIM],
                        in_=w_in[dt * 128:(dt + 1) * 128, :])
                nc.vector.tensor_copy(w_r, w_f)
                wo_f = wload.tile([128, D], F32)
                nc.sync.dma_start(out=wo_f, in_=wo_in[:, :])
                nc.vector.tensor_copy(woA_r, wo_f[0:64, :])
                # bring rows 64:127 down to partitions 0:63 via sbuf-sbuf dma
                woB_f = wload.tile([64, D], F32)
                nc.sync.dma_start(out=woB_f, in_=wo_f[64:128, :])
                nc.vector.tensor_copy(woB_r, woB_f)

                # ---- phase 1: QKV projection ----
                with tc.tile_pool(name="xst", bufs=6) as xst_pool, \
                     tc.tile_pool(name="xtp", bufs=16) as xt_pool, \
                     tc.tile_pool(name="ps1t", bufs=2, space="PSUM") as ps1t, \
                     tc.tile_pool(name="ps1q", bufs=3, space="PSUM") as ps1q:
                    for ch in range(NCH):
                        xsts = []
                        for i in range(4):
                            xt_st = xst_pool.tile([128, D], F32, tag="xst")
                            nc.sync.dma_start(
                                out=xt_st,
                                in_=x_in[ch * 512 + i * 128:
                                         ch * 512 + (i + 1) * 128, :])
                            xsts.append(xt_st)
                        xts = []
                        for dt in range(NDT):
                            tps = ps1t.tile([128, 512], F32, tag="tps")
                            for i in range(4):
                                nc.tensor.transpose(
                                    tps[:, i * 128:(i + 1) * 128],
                                    xsts[i][:, dt * 128:(dt + 1) * 128],
                                    ident)
                            xt = xt_pool.tile([128, 512], F32R, tag="xt")
                            nc.vector.tensor_copy(xt, tps)
                            xts.append(xt)
                        for f, ft in enumerate((ftq, ftk, ftv)):
                            qps = ps1q.tile([128, 512], F32, tag="qps")
                            for dt in range(NDT):
                                nc.tensor.matmul(
                                    qps,
                                    w_r[:, dt * FDIM + f * 128:
                                        dt * FDIM + (f + 1) * 128],
                                    xts[dt],
                                    start=(dt == 0), stop=(dt == NDT - 1))
                            nc.vector.tensor_scalar(
                                out=ft[:, ch * 512:(ch + 1) * 512],
                                in0=qps, scalar1=bias_sb[:, f:f + 1],
                                scalar2=None, op0=mybir.AluOpType.add)

            # ---- phase 1.5: build augmented v (token-major + ones col) ----
            with tc.tile_pool(name="ps_vt", bufs=3, space="PSUM") as ps_vt:
                for b in range(B):
                    for h in range(2):
                        va = vall[b][h]
                        nc.vector.tensor_copy(
                            va.rearrange("p (j c) -> p j c", c=65)[:, :, 64],
                            onescol_f.broadcast_to([128, NJT]))
                    for jt in range(NJT):
                        tp = ps_vt.tile([128, 128], F32, tag="vt")
                        nc.tensor.transpose(
                            tp, ftv[:, b * S + jt * 128: b * S + (jt + 1) * 128],
                            ident)
                        for h in range(2):
                            nc.vector.tensor_copy(
                                vall[b][h].rearrange(
                                    "p (j c) -> p j c", c=65)[:, jt, 0:64],
                                tp[:, h * 64:(h + 1) * 64])

            # ---- phase 2: attention ----
            with tc.tile_pool(name="ps_sc", bufs=2, space="PSUM") as ps_sc, \
                 tc.tile_pool(name="ps_oT", bufs=2, space="PSUM") as ps_oT, \
                 tc.tile_pool(name="ptp", bufs=3) as ptp, \
                 tc.tile_pool(name="evp", bufs=4) as evp, \
                 tc.tile_pool(name="denp", bufs=2) as denp:
                for b in range(B):
                    evs = {}
                    denA = denp.tile([NSC, 512], F32, tag="denA")
                    denB = denp.tile([NSC, 512], F32, tag="denB")
                    for sc in range(NSC):
                        oTa = ps_oT.tile([65, 512], F32, tag="oTa")
                        oTb = ps_oT.tile([65, 512], F32, tag="oTb")
                        q0 = b * S + sc * 512
                        for jt in range(NJT):
                            k0 = b * S + jt * 128
                            sps = ps_sc.tile([128, 1024], F32, tag="sps")
                            nc.tensor.matmul(
                                sps[:, 0:512],
                                ftk[0:64, k0:k0 + 128],
                                ftq[0:64, q0:q0 + 512],
                                start=True, stop=True, tile_position=(0, 0))
                            nc.tensor.matmul(
                                sps[:, 512:1024],
                                ftk[64:128, k0:k0 + 128],
                                ftq[64:128, q0:q0 + 512],
                                start=True, stop=True, tile_position=(64, 0))
                            pt = ptp.tile([128, 1024], F32R, tag="pt")
                            nc.scalar.activation(out=pt, in_=sps, func=ACT.Exp,
                                                 scale=0.125)
                            nc.tensor.matmul(
                                oTa, vall[b][0][:, jt * 65:(jt + 1) * 65],
                                pt[:, 0:512],
                                start=(jt == 0), stop=(jt == NJT - 1))
                            nc.tensor.matmul(
                                oTb, vall[b][1][:, jt * 65:(jt + 1) * 65],
                                pt[:, 512:1024],
                                start=(jt == 0), stop=(jt == NJT - 1))
                        # evacuate unnormalized o^T (+ den row) to SBUF
                        evA = evp.tile([65, 512], F32, tag="evA")
                        nc.vector.tensor_copy(evA, oTa)
                        evB = evp.tile([65, 512], F32, tag="evB")
                        nc.vector.tensor_copy(evB, oTb)
                        evs[(sc, 0)] = evA
                        evs[(sc, 1)] = evB
                        # den rows -> partition sc of den tiles (sbuf-sbuf dma)
                        nc.sync.dma_start(out=denA[sc:sc + 1, :],
                                          in_=evA[64:65, :])
                        nc.sync.dma_start(out=denB[sc:sc + 1, :],
                                          in_=evB[64:65, :])
                    # reciprocal of denominators: 1/x = exp(-ln(x))
                    recs = []
                    for head, den in ((0, denA), (1, denB)):
                        lnd = denp.tile([NSC, 512], F32, tag="lnd")
                        nc.scalar.activation(out=lnd, in_=den, func=ACT.Ln)
                        rec = denp.tile([NSC, 512], F32R, tag="rec")
                        nc.scalar.activation(out=rec, in_=lnd, func=ACT.Exp,
                                             scale=-1.0)
                        recs.append(rec)
                    # normalize: oT_x[:, cols] = ev * bcast(rec)
                    for sc in range(NSC):
                        for head, oT_full in ((0, oT_A), (1, oT_B)):
                            rec = recs[head]
                            rec0 = denp.tile([1, 512], F32R, tag="rec0")
                            nc.sync.dma_start(out=rec0, in_=rec[sc:sc + 1, :])
                            bcp = ps_oT.tile([64, 512], F32, tag="oTa")
                            nc.tensor.matmul(bcp, ones1, rec0,
                                             start=True, stop=True)
                            nc.vector.tensor_mul(
                                oT_full[:, b * S + sc * 512:
                                        b * S + (sc + 1) * 512],
                                evs[(sc, head)][0:64, :], bcp)

            # ---- phase 3: output projection (partial y) ----
            with tc.tile_pool(name="ps_y", bufs=2, space="PSUM") as ps_y, \
                 tc.tile_pool(name="ysb", bufs=3) as ysb:
                for tt in range(TOK // 128):
                    yps = ps_y.tile([128, 1024], F32, tag="y")
                    for n in range(2):
                        nc.tensor.matmul(
                            yps[:, n * 512:(n + 1) * 512],
                            oT_A[:, tt * 128:(tt + 1) * 128],
                            woA_r[:, n * 512:(n + 1) * 512],
                            start=True, stop=False)
                        nc.tensor.matmul(
                            yps[:, n * 512:(n + 1) * 512],
                            oT_B[:, tt * 128:(tt + 1) * 128],
                            woB_r[:, n * 512:(n + 1) * 512],
                            start=False, stop=True)
                    ysb_t = ysb.tile([128, 1024], F32, tag="ysb")
                    nc.vector.tensor_copy(ysb_t, yps)
                    nc.sync.dma_start(
                        out=y_out[tt * 128:(tt + 1) * 128, :], in_=ysb_t)

    _split_multi_waits(nc)
    return nc


_NC_CACHE = None


def _get_nc():
    global _NC_CACHE
    if _NC_CACHE is None:
        _NC_CACHE = build_bass()
    return _NC_CACHE


def make_in_maps(x, Wq, bq, Wk, bk, Wv, bv, Wo, bo):
    x2 = np.ascontiguousarray(np.asarray(x, np.float32).reshape(TOK, D))
    in_maps = []
    for c in range(NCORES):
        h0, h1 = 2 * c, 2 * c + 1
        w_c = np.concatenate(
            [Wq[h0], Wq[h1], Wk[h0], Wk[h1], Wv[h0], Wv[h1]], axis=1)
        b_c = np.concatenate(
            [bq[h0], bq[h1], bk[h0], bk[h1], bv[h0], bv[h1]])
        wo_c = Wo[c * 128:(c + 1) * 128, :]
        in_maps.append({
            "x": x2,
            "w": np.ascontiguousarray(w_c, dtype=np.float32),
            "b": np.ascontiguousarray(b_c.reshape(FDIM, 1), dtype=np.float32),
            "wo": np.ascontiguousarray(wo_c, dtype=np.float32),
        })
    return in_maps


def kernel(x, Wq, bq, Wk, bk, Wv, bv, Wo, bo):
    nc = _get_nc()
    in_maps = make_in_maps(x, Wq, bq, Wk, bk, Wv, bv, Wo, bo)
    last_err = None
    for attempt in range(8):
        try:
            res = run_bass_kernel_spmd(nc, in_maps, list(range(NCORES)))
            break
        except Exception as e:
            msg = str(e)
            if "UNAVAILABLE" in msg or "unrecoverable" in msg:
                last_err = e
                time.sleep(45)
                continue
            raise
    else:
        raise RuntimeError(f"device never recovered: {last_err}")
    y = np.zeros((TOK, D), np.float64)
    for c in range(NCORES):
        y += res.results[c]["y"].astype(np.float64)
    y += np.asarray(bo, np.float64)[None, :]
    return y.reshape(B, S, D).astype(np.float32)
